# revision 1
# baseline (speedup 1.0000x reference)
"""Locally-connected graph-conv kernel for Trainium2 (Bass/Tile).

Computes out[b,t,m] = sum_n x[b,t,n] * (S*W)[n,m] + bias[m] for
x [64, 2048, 208], W/S [208, 208], bias [208].

The ring-graph support S is a +-4 band (mod 208), so each half of the
output nodes only needs a 112-row slice of the contraction dim:
  block 0 (m 0..103):   n in {204..207} ++ {0..107}
  block 1 (m 104..207): n in {100..207} ++ {0..3}
Each output block is then a SINGLE [112,104] x [112,512] fp32 matmul with
the masked-weight block stationary in the PE array and x^T streaming as
the moving operand in 512-column blocks (long streams hide the fp32
LDWEIGHTS). The bias is fused into the PSUM->SBUF eviction as a
per-partition tensor_scalar add on VectorE.

Data-parallel over 8 NeuronCores: each core gets 16384 rows of the
flattened x, host-pre-assembled into a [224, 16384] tensor (two 112-row
halo blocks). DMA partition counts are multiples of 16 (the fast HWDGE
path: ~250 GB/s/instr vs ~27 otherwise); stores are [112]-row DMAs into
a [224, SHARD] output (8 pad rows per block, dropped at host gather).
x loads issue on the Sync HWDGE ring, stores on the Scalar ring, one-time
weight/bias setup on the GpSimd SWDGE queue so it never delays them.
The host transposes y^T back at gather.
"""

import numpy as np
from contextlib import ExitStack

import concourse.bacc as bacc
import concourse.mybir as mybir
import concourse.tile as tile
from concourse.bass_utils import run_bass_kernel_spmd

N = 208                      # nodes
HALF = 104                   # output nodes per block
K = 4                        # band half-width of S
NH = 2 * K + HALF            # 112 contraction rows per block (halo incl.)
NP = 112                     # padded store rows (multiple of 16)
N_CORES = 8
B, T = 64, 2048
ROWS_TOTAL = B * T           # 131072
SHARD = ROWS_TOTAL // N_CORES    # 16384 rows per core
TB = 512                     # moving-block columns per matmul (fp32 PSUM max)
TB2 = 2 * TB                 # eviction group (2 PSUM banks)
TOUT = 2048                  # t-columns per DMA chunk (~0.9 MB loads)
N_CHUNKS = SHARD // TOUT     # 8
SUB = TOUT // TB2            # 2 psum groups per chunk

FP32 = mybir.dt.float32

# halo row order (indices into the [208] node dim) for each block
ROWS0 = list(range(N - K, N)) + list(range(0, HALF + K))          # 112
ROWS1 = list(range(HALF - K, N)) + list(range(0, K))              # 112

_CACHE = {}
LAST_RESULTS = None          # BassKernelResults of the most recent run


def _kernel_body(tc):
    nc = tc.nc
    # rows 0:112 block0 halo, 112:224 block1 halo
    x_d = nc.dram_tensor("xh", [2 * NH, SHARD], FP32, kind="ExternalInput").ap()
    w_d = nc.dram_tensor("w", [N, N], FP32, kind="ExternalInput").ap()
    s_d = nc.dram_tensor("s", [N, N], FP32, kind="ExternalInput").ap()
    b_d = nc.dram_tensor("bias", [1, N], FP32, kind="ExternalInput").ap()
    o_d = nc.dram_tensor("outt", [2 * NP, SHARD], FP32, kind="ExternalOutput").ap()

    with ExitStack() as ctx:
        const = ctx.enter_context(tc.tile_pool(name="const", bufs=1))

        # One-time setup: w/s pieces on the Scalar HWDGE ring (fast issue,
        # idle at startup), bias on GpSimd. Stationary blocks wh0/wh1
        # [112, 104]: masked weight rows in halo order. Bias [104, 1].
        w0 = const.tile([NH, HALF], FP32, tag="w0")
        s0 = const.tile([NH, HALF], FP32, tag="s0")
        nc.scalar.dma_start(w0[0:K, :], w_d[N - K : N, 0:HALF])
        nc.scalar.dma_start(w0[K:NH, :], w_d[0 : HALF + K, 0:HALF])
        nc.scalar.dma_start(s0[0:K, :], s_d[N - K : N, 0:HALF])
        nc.scalar.dma_start(s0[K:NH, :], s_d[0 : HALF + K, 0:HALF])
        wh0 = const.tile([NH, HALF], FP32, tag="wh0")
        nc.vector.tensor_mul(wh0, w0, s0)
        w1 = const.tile([NH, HALF], FP32, tag="w1")
        s1 = const.tile([NH, HALF], FP32, tag="s1")
        nc.scalar.dma_start(w1[0 : HALF + K, :], w_d[HALF - K : N, HALF:N])
        nc.scalar.dma_start(w1[HALF + K : NH, :], w_d[0:K, HALF:N])
        nc.scalar.dma_start(s1[0 : HALF + K, :], s_d[HALF - K : N, HALF:N])
        nc.scalar.dma_start(s1[HALF + K : NH, :], s_d[0:K, HALF:N])
        wh1 = const.tile([NH, HALF], FP32, tag="wh1")
        nc.vector.tensor_mul(wh1, w1, s1)
        bA = const.tile([HALF, 1], FP32, tag="bA")
        bB = const.tile([HALF, 1], FP32, tag="bB")
        b_col = b_d.rearrange("o n -> n o")
        nc.gpsimd.dma_start(bA, b_col[0:HALF, :])
        nc.gpsimd.dma_start(bB, b_col[HALF:N, :])

        x0p = ctx.enter_context(tc.tile_pool(name="x0p", bufs=6))
        x1p = ctx.enter_context(tc.tile_pool(name="x1p", bufs=6))
        o0p = ctx.enter_context(tc.tile_pool(name="o0p", bufs=4))
        o1p = ctx.enter_context(tc.tile_pool(name="o1p", bufs=4))
        ps0p = ctx.enter_context(tc.tile_pool(name="ps0p", bufs=2, space="PSUM"))
        ps1p = ctx.enter_context(tc.tile_pool(name="ps1p", bufs=2, space="PSUM"))

        for c in range(N_CHUNKS):
            tsl = slice(c * TOUT, (c + 1) * TOUT)
            xh0 = x0p.tile([NH, TOUT], FP32, tag="xh0")
            xh1 = x1p.tile([NH, TOUT], FP32, tag="xh1")
            if c == 0:
                # split the critical-path first loads for 2x DMA concurrency
                nc.sync.dma_start(xh0[0:64, :], x_d[0:64, tsl])
                nc.sync.dma_start(xh0[64:NH, :], x_d[64:NH, tsl])
                nc.sync.dma_start(xh1[0:64, :], x_d[NH : NH + 64, tsl])
                nc.sync.dma_start(xh1[64:NH, :], x_d[NH + 64 : 2 * NH, tsl])
            else:
                nc.sync.dma_start(xh0, x_d[0:NH, tsl])
                nc.sync.dma_start(xh1, x_d[NH : 2 * NH, tsl])

            o0_t = o0p.tile([NP, TOUT], FP32, tag="o0")
            o1_t = o1p.tile([NP, TOUT], FP32, tag="o1")
            for s in range(SUB):
                g = slice(s * TB2, (s + 1) * TB2)
                ga = slice(s * TB2, s * TB2 + TB)
                gb = slice(s * TB2 + TB, (s + 1) * TB2)
                # [104, 1024] PSUM tiles (2 banks); each matmul fills one bank
                ps0 = ps0p.tile([HALF, TB2], FP32, tag="ps0")
                nc.tensor.matmul(ps0[:, 0:TB], wh0, xh0[:, ga], start=True, stop=True)
                nc.tensor.matmul(ps0[:, TB:TB2], wh0, xh0[:, gb], start=True, stop=True)
                ps1 = ps1p.tile([HALF, TB2], FP32, tag="ps1")
                nc.tensor.matmul(ps1[:, 0:TB], wh1, xh1[:, ga], start=True, stop=True)
                nc.tensor.matmul(ps1[:, TB:TB2], wh1, xh1[:, gb], start=True, stop=True)
                # eviction + per-partition bias on VectorE
                nc.vector.tensor_scalar_add(o0_t[0:HALF, g], ps0, bA)
                nc.vector.tensor_scalar_add(o1_t[0:HALF, g], ps1, bB)
            # per-chunk stores (112 rows, 8 pad) on the Scalar HWDGE ring;
            # the last chunk's second-block store rides the by-then-idle Sync
            # ring so the two tail stores run in parallel
            nc.scalar.dma_start(o_d[0:NP, tsl], o0_t)
            if c == N_CHUNKS - 1:
                nc.sync.dma_start(o_d[NP : 2 * NP, tsl], o1_t)
            else:
                nc.scalar.dma_start(o_d[NP : 2 * NP, tsl], o1_t)


def _build():
    nc = bacc.Bacc(
        "TRN2",
        target_bir_lowering=False,
        debug=False,
        num_devices=N_CORES,
    )
    with tile.TileContext(nc) as tc:
        _kernel_body(tc)
    nc.compile()
    return nc


def kernel(x, W, b, S):
    global LAST_RESULTS
    nc = _CACHE.get("nc")
    if nc is None:
        nc = _build()
        _CACHE["nc"] = nc

    xf = np.asarray(x, np.float32).reshape(ROWS_TOTAL, N)
    Wf = np.ascontiguousarray(np.asarray(W, np.float32))
    Sf = np.ascontiguousarray(np.asarray(S, np.float32))
    bf = np.ascontiguousarray(np.asarray(b, np.float32).reshape(1, N))

    in_maps = []
    for i in range(N_CORES):
        xt = xf[i * SHARD : (i + 1) * SHARD].T          # [208, SHARD] view
        xh = np.empty((2 * NH, SHARD), np.float32)
        xh[0:NH] = xt[ROWS0]
        xh[NH : 2 * NH] = xt[ROWS1]
        in_maps.append({"xh": xh, "w": Wf, "s": Sf, "bias": bf})
    res = run_bass_kernel_spmd(nc, in_maps, core_ids=list(range(N_CORES)))
    LAST_RESULTS = res
    out = np.empty((ROWS_TOTAL, N), np.float32)
    for i, r in enumerate(res.results):
        yt = r["outt"]                                  # [224, SHARD]
        out[i * SHARD : (i + 1) * SHARD, 0:HALF] = yt[0:HALF].T
        out[i * SHARD : (i + 1) * SHARD, HALF:N] = yt[NP : NP + HALF].T
    return out.reshape(B, T, N)



# revision 2
# speedup vs baseline: 1.7702x; 1.7702x over previous
"""Locally-connected graph-conv kernel for Trainium2 (Bass/Tile).

Computes out[b,t,m] = sum_n x[b,t,n] * (S*W)[n,m] + bias[m] for
x [64, 2048, 208], W/S [208, 208], bias [208].

The ring-graph support S is a +-4 band (mod 208), so each half of the
output nodes only needs a 112-row slice of the contraction dim:
  block 0 (m 0..103):   n in {204..207} ++ {0..107}
  block 1 (m 104..207): n in {100..207} ++ {0..3}
Each output block is a [112,104] x [112,512] matmul with the masked
weight block stationary and x^T streaming as the moving operand.

Memory-bound problem, so everything streams in bf16: the host casts x
and the pre-masked weights S*W to bf16 (well inside the 2e-2 rel-err
envelope), matmuls accumulate in fp32 PSUM, and the PSUM->SBUF
eviction converts to bf16 for the store, halving HBM traffic in both
directions. Evictions are split across engines so neither becomes the
bottleneck at bf16 speed: block 0 on VectorE (tensor_scalar add of the
per-partition bias), block 1 on the Activation engine (Identity
activation with bias AP).

Data-parallel over 8 NeuronCores: each core gets 16384 rows of the
flattened x, host-pre-assembled into a [224, 16384] bf16 tensor (two
112-row halo blocks; partition counts stay multiples of 16 for the
fast HWDGE path). x loads issue on the Sync HWDGE ring, block-0 stores
on the Scalar ring, block-1 stores on the GpSimd SWDGE queue, so loads
never queue behind stores. The host transposes y^T back at gather.
"""

import numpy as np
import ml_dtypes
from contextlib import ExitStack

import concourse.bacc as bacc
import concourse.mybir as mybir
import concourse.tile as tile
from concourse.bass_utils import run_bass_kernel_spmd

N = 208                      # nodes
HALF = 104                   # output nodes per block
K = 4                        # band half-width of S
NH = 2 * K + HALF            # 112 contraction rows per block (halo incl.)
NP = 112                     # padded store rows (multiple of 16)
N_CORES = 8
B, T = 64, 2048
ROWS_TOTAL = B * T           # 131072
SHARD = ROWS_TOTAL // N_CORES    # 16384 rows per core
TB = 512                     # moving-block columns per matmul (fp32 PSUM max)
TB2 = 2 * TB                 # eviction group (2 PSUM banks)
TOUT = 2048                  # t-columns per DMA chunk
N_CHUNKS = SHARD // TOUT     # 8
SUB = TOUT // TB2            # 2 psum groups per chunk

FP32 = mybir.dt.float32
BF16 = mybir.dt.bfloat16
NP_BF16 = np.dtype(ml_dtypes.bfloat16)

# halo row order (indices into the [208] node dim) for each block
ROWS0 = list(range(N - K, N)) + list(range(0, HALF + K))          # 112
ROWS1 = list(range(HALF - K, N)) + list(range(0, K))              # 112

_CACHE = {}
LAST_RESULTS = None          # BassKernelResults of the most recent run


def _kernel_body(tc):
    nc = tc.nc
    # rows 0:112 block0 halo, 112:224 block1 halo
    x_d = nc.dram_tensor("xh", [2 * NH, SHARD], BF16, kind="ExternalInput").ap()
    w_d = nc.dram_tensor("wh", [NH, N], BF16, kind="ExternalInput").ap()
    b_d = nc.dram_tensor("bias", [HALF, 2], FP32, kind="ExternalInput").ap()
    o_d = nc.dram_tensor("outt", [2 * NP, SHARD], BF16, kind="ExternalOutput").ap()

    with ExitStack() as ctx:
        const = ctx.enter_context(tc.tile_pool(name="const", bufs=1))

        # One-time setup on the Scalar HWDGE ring (idle at startup):
        # host-pre-masked stationary weight blocks wh0/wh1 [112, 104] in
        # halo row order, bias columns [104, 1] per block.
        wh = const.tile([NH, N], BF16, tag="wh")
        nc.scalar.dma_start(wh, w_d)
        bcols = const.tile([HALF, 2], FP32, tag="bcols")
        nc.scalar.dma_start(bcols, b_d)
        wh0 = wh[:, 0:HALF]
        wh1 = wh[:, HALF:N]
        bA = bcols[:, 0:1]
        bB = bcols[:, 1:2]

        x0p = ctx.enter_context(tc.tile_pool(name="x0p", bufs=6))
        x1p = ctx.enter_context(tc.tile_pool(name="x1p", bufs=6))
        o0p = ctx.enter_context(tc.tile_pool(name="o0p", bufs=4))
        o1p = ctx.enter_context(tc.tile_pool(name="o1p", bufs=4))
        ps0p = ctx.enter_context(tc.tile_pool(name="ps0p", bufs=2, space="PSUM"))
        ps1p = ctx.enter_context(tc.tile_pool(name="ps1p", bufs=2, space="PSUM"))

        for c in range(N_CHUNKS):
            tsl = slice(c * TOUT, (c + 1) * TOUT)
            xh0 = x0p.tile([NH, TOUT], BF16, tag="xh0")
            xh1 = x1p.tile([NH, TOUT], BF16, tag="xh1")
            if c == 0:
                # split the critical-path first loads for 2x DMA concurrency
                nc.sync.dma_start(xh0[0:64, :], x_d[0:64, tsl])
                nc.sync.dma_start(xh0[64:NH, :], x_d[64:NH, tsl])
                nc.sync.dma_start(xh1[0:64, :], x_d[NH : NH + 64, tsl])
                nc.sync.dma_start(xh1[64:NH, :], x_d[NH + 64 : 2 * NH, tsl])
            else:
                nc.sync.dma_start(xh0, x_d[0:NH, tsl])
                nc.sync.dma_start(xh1, x_d[NH : 2 * NH, tsl])

            o0_t = o0p.tile([NP, TOUT], BF16, tag="o0")
            o1_t = o1p.tile([NP, TOUT], BF16, tag="o1")
            for s in range(SUB):
                g = slice(s * TB2, (s + 1) * TB2)
                ga = slice(s * TB2, s * TB2 + TB)
                gb = slice(s * TB2 + TB, (s + 1) * TB2)
                # [104, 1024] PSUM tiles (2 banks); each matmul fills one bank
                ps0 = ps0p.tile([HALF, TB2], FP32, tag="ps0")
                nc.tensor.matmul(ps0[:, 0:TB], wh0, xh0[:, ga], start=True, stop=True)
                nc.tensor.matmul(ps0[:, TB:TB2], wh0, xh0[:, gb], start=True, stop=True)
                ps1 = ps1p.tile([HALF, TB2], FP32, tag="ps1")
                nc.tensor.matmul(ps1[:, 0:TB], wh1, xh1[:, ga], start=True, stop=True)
                nc.tensor.matmul(ps1[:, TB:TB2], wh1, xh1[:, gb], start=True, stop=True)
                # eviction + per-partition bias, split across engines:
                # block 0 on VectorE, block 1 on the Activation engine
                nc.vector.tensor_scalar_add(o0_t[0:HALF, g], ps0, bA)
                nc.scalar.add(o1_t[0:HALF, g], ps1, bB)
            # block-0 stores ride the Scalar HWDGE ring, block-1 stores the
            # GpSimd SWDGE queue; loads keep the Sync ring to themselves
            nc.scalar.dma_start(o_d[0:NP, tsl], o0_t)
            nc.gpsimd.dma_start(o_d[NP : 2 * NP, tsl], o1_t)


def _build():
    nc = bacc.Bacc(
        "TRN2",
        target_bir_lowering=False,
        debug=False,
        num_devices=N_CORES,
    )
    with tile.TileContext(nc) as tc:
        _kernel_body(tc)
    nc.compile()
    return nc


def kernel(x, W, b, S):
    global LAST_RESULTS
    nc = _CACHE.get("nc")
    if nc is None:
        nc = _build()
        _CACHE["nc"] = nc

    xf = np.asarray(x, np.float32).reshape(ROWS_TOTAL, N)
    Mf = np.asarray(S, np.float32) * np.asarray(W, np.float32)
    Mh = Mf.astype(NP_BF16)
    wh = np.empty((NH, N), NP_BF16)
    wh[:, 0:HALF] = Mh[ROWS0][:, 0:HALF]
    wh[:, HALF:N] = Mh[ROWS1][:, HALF:N]
    bf = np.asarray(b, np.float32)
    bcols = np.empty((HALF, 2), np.float32)
    bcols[:, 0] = bf[0:HALF]
    bcols[:, 1] = bf[HALF:N]

    xb = xf.astype(NP_BF16)
    in_maps = []
    for i in range(N_CORES):
        xt = xb[i * SHARD : (i + 1) * SHARD].T          # [208, SHARD] view
        xh = np.empty((2 * NH, SHARD), NP_BF16)
        xh[0:NH] = xt[ROWS0]
        xh[NH : 2 * NH] = xt[ROWS1]
        in_maps.append({"xh": xh, "wh": wh, "bias": bcols})
    res = run_bass_kernel_spmd(nc, in_maps, core_ids=list(range(N_CORES)))
    LAST_RESULTS = res
    out = np.empty((ROWS_TOTAL, N), np.float32)
    for i, r in enumerate(res.results):
        yt = r["outt"]                                  # [224, SHARD] bf16
        out[i * SHARD : (i + 1) * SHARD, 0:HALF] = yt[0:HALF].T.astype(np.float32)
        out[i * SHARD : (i + 1) * SHARD, HALF:N] = yt[NP : NP + HALF].T.astype(
            np.float32
        )
    return out.reshape(B, T, N)


# revision 5
# speedup vs baseline: 1.7796x; 1.0053x over previous
"""Locally-connected graph-conv kernel for Trainium2 (Bass/Tile).

Computes out[b,t,m] = sum_n x[b,t,n] * (S*W)[n,m] + bias[m] for
x [64, 2048, 208], W/S [208, 208], bias [208].

The ring-graph support S is a +-4 band (mod 208), so each half of the
output nodes only needs a 112-row slice of the contraction dim:
  block 0 (m 0..103):   n in {204..207} ++ {0..107}
  block 1 (m 104..207): n in {100..207} ++ {0..3}
Each output block is a [112,104] x [112,512] matmul with the masked
weight block stationary and x^T streaming as the moving operand.

Memory-bound, so everything streams in bf16 (host casts x and the
pre-masked S*W; PSUM accumulates fp32; the eviction converts back to
bf16), halving HBM traffic both ways and staying well inside the 2e-2
rel-err envelope.

Pipeline structure (per NeuronCore, tuned from the perfetto trace):
 - Column chunks are TAPERED [512,512,1024, 2048 x6, 1024,512,512] so
   the first matmul starts ~3us earlier and the serial drain after the
   last load (matmul->evict->store) shrinks from ~14us to ~5us.
 - Loads keep the Sync HWDGE ring to themselves (only SP/Act/Pool can
   issue DMAs); load emission is hoisted 3 chunks ahead of compute.
 - PSUM->SBUF evictions split per block: block0 on VectorE
   (tensor_scalar add of bias), block1 on the Activation engine
   (Identity+bias). Stores issue at PSUM-group granularity right
   after each eviction: block1 on the Scalar ring (its own engine's
   product, no foreign deps), block0 on the GpSimd SWDGE queue.

Data-parallel over 8 NeuronCores: each core gets 16384 rows of the
flattened x, host-pre-assembled into a [224, 16384] bf16 tensor (two
112-row halo blocks; partition counts multiples of 16 for the fast
HWDGE path). The host transposes y^T back at gather.
"""

import numpy as np
import ml_dtypes
from contextlib import ExitStack

import concourse.bacc as bacc
import concourse.mybir as mybir
import concourse.tile as tile
from concourse.bass_utils import run_bass_kernel_spmd

N = 208                      # nodes
HALF = 104                   # output nodes per block
K = 4                        # band half-width of S
NH = 2 * K + HALF            # 112 contraction rows per block (halo incl.)
NP = 112                     # padded store rows (multiple of 16)
N_CORES = 8
B, T = 64, 2048
ROWS_TOTAL = B * T           # 131072
SHARD = ROWS_TOTAL // N_CORES    # 16384 rows per core
TB = 512                     # moving-block columns per matmul (fp32 PSUM max)
GMAX = 1024                  # eviction/store group (2 PSUM banks)
AHEAD = 3                    # chunks of load prefetch hoisted past compute

# tapered chunk schedule (columns per chunk), sums to SHARD
CHUNKS = [512, 512, 1024] + [2048] * 6 + [1024, 512, 512]
assert sum(CHUNKS) == SHARD

FP32 = mybir.dt.float32
BF16 = mybir.dt.bfloat16
NP_BF16 = np.dtype(ml_dtypes.bfloat16)

# halo row order (indices into the [208] node dim) for each block
ROWS0 = list(range(N - K, N)) + list(range(0, HALF + K))          # 112
ROWS1 = list(range(HALF - K, N)) + list(range(0, K))              # 112

_CACHE = {}
LAST_RESULTS = None          # BassKernelResults of the most recent run


def _kernel_body(tc):
    nc = tc.nc
    # rows 0:112 block0 halo, 112:224 block1 halo
    x_d = nc.dram_tensor("xh", [2 * NH, SHARD], BF16, kind="ExternalInput").ap()
    w_d = nc.dram_tensor("wh", [NH, N], BF16, kind="ExternalInput").ap()
    b_d = nc.dram_tensor("bias", [HALF, 2], FP32, kind="ExternalInput").ap()
    o_d = nc.dram_tensor("outt", [2 * NP, SHARD], BF16, kind="ExternalOutput").ap()

    starts = [sum(CHUNKS[:i]) for i in range(len(CHUNKS))]
    NCH = len(CHUNKS)

    with ExitStack() as ctx:
        const = ctx.enter_context(tc.tile_pool(name="const", bufs=1))

        # One-time setup on the Scalar HWDGE ring (idle at startup):
        # host-pre-masked stationary weight blocks wh0/wh1 [112, 104] in
        # halo row order, bias columns [104, 1] per block.
        wh = const.tile([NH, N], BF16, tag="wh")
        nc.scalar.dma_start(wh, w_d)
        bcols = const.tile([HALF, 2], FP32, tag="bcols")
        nc.scalar.dma_start(bcols, b_d)
        wh0 = wh[:, 0:HALF]
        wh1 = wh[:, HALF:N]
        bA = bcols[:, 0:1]
        bB = bcols[:, 1:2]

        x0p = ctx.enter_context(tc.tile_pool(name="x0p", bufs=6))
        x1p = ctx.enter_context(tc.tile_pool(name="x1p", bufs=6))
        o0p = ctx.enter_context(tc.tile_pool(name="o0p", bufs=4))
        o1p = ctx.enter_context(tc.tile_pool(name="o1p", bufs=4))
        ps0p = ctx.enter_context(tc.tile_pool(name="ps0p", bufs=2, space="PSUM"))
        ps1p = ctx.enter_context(tc.tile_pool(name="ps1p", bufs=2, space="PSUM"))

        xtiles = {}

        def emit_load(c):
            clen = CHUNKS[c]
            tsl = slice(starts[c], starts[c] + clen)
            xh0 = x0p.tile([NH, 2048], BF16, tag="xh0")
            xh1 = x1p.tile([NH, 2048], BF16, tag="xh1")
            nc.sync.dma_start(xh0[:, 0:clen], x_d[0:NH, tsl])
            nc.sync.dma_start(xh1[:, 0:clen], x_d[NH : 2 * NH, tsl])
            xtiles[c] = (xh0, xh1)

        for c in range(min(AHEAD, NCH)):
            emit_load(c)

        for c in range(NCH):
            clen = CHUNKS[c]
            cstart = starts[c]
            xh0, xh1 = xtiles.pop(c)
            for g0 in range(0, clen, GMAX):
                glen = min(GMAX, clen - g0)
                gsl = slice(cstart + g0, cstart + g0 + glen)
                ps0 = ps0p.tile([HALF, GMAX], FP32, tag="ps0")
                ps1 = ps1p.tile([HALF, GMAX], FP32, tag="ps1")
                for m0 in range(0, glen, TB):
                    mlen = min(TB, glen - m0)
                    msl = slice(g0 + m0, g0 + m0 + mlen)
                    psl = slice(m0, m0 + mlen)
                    nc.tensor.matmul(
                        ps0[:, psl], wh0, xh0[:, msl], start=True, stop=True
                    )
                    nc.tensor.matmul(
                        ps1[:, psl], wh1, xh1[:, msl], start=True, stop=True
                    )
                # eviction + per-partition bias, split across engines;
                # each block's store issues right after its eviction
                o0_t = o0p.tile([NP, GMAX], BF16, tag="o0")
                o1_t = o1p.tile([NP, GMAX], BF16, tag="o1")
                nc.vector.tensor_scalar_add(
                    o0_t[0:HALF, 0:glen], ps0[:, 0:glen], bA
                )
                nc.scalar.add(o1_t[0:HALF, 0:glen], ps1[:, 0:glen], bB)
                nc.gpsimd.dma_start(o_d[0:NP, gsl], o0_t[:, 0:glen])
                nc.scalar.dma_start(o_d[NP : 2 * NP, gsl], o1_t[:, 0:glen])
            if c + AHEAD < NCH:
                emit_load(c + AHEAD)


def _build():
    nc = bacc.Bacc(
        "TRN2",
        target_bir_lowering=False,
        debug=False,
        num_devices=N_CORES,
    )
    with tile.TileContext(nc) as tc:
        _kernel_body(tc)
    nc.compile()
    return nc


def kernel(x, W, b, S):
    global LAST_RESULTS
    nc = _CACHE.get("nc")
    if nc is None:
        nc = _build()
        _CACHE["nc"] = nc

    xf = np.asarray(x, np.float32).reshape(ROWS_TOTAL, N)
    Mf = np.asarray(S, np.float32) * np.asarray(W, np.float32)
    Mh = Mf.astype(NP_BF16)
    wh = np.empty((NH, N), NP_BF16)
    wh[:, 0:HALF] = Mh[ROWS0][:, 0:HALF]
    wh[:, HALF:N] = Mh[ROWS1][:, HALF:N]
    bf = np.asarray(b, np.float32)
    bcols = np.empty((HALF, 2), np.float32)
    bcols[:, 0] = bf[0:HALF]
    bcols[:, 1] = bf[HALF:N]

    xb = xf.astype(NP_BF16)
    in_maps = []
    for i in range(N_CORES):
        xt = xb[i * SHARD : (i + 1) * SHARD].T          # [208, SHARD] view
        xh = np.empty((2 * NH, SHARD), NP_BF16)
        xh[0:NH] = xt[ROWS0]
        xh[NH : 2 * NH] = xt[ROWS1]
        in_maps.append({"xh": xh, "wh": wh, "bias": bcols})
    res = run_bass_kernel_spmd(nc, in_maps, core_ids=list(range(N_CORES)))
    LAST_RESULTS = res
    out = np.empty((ROWS_TOTAL, N), np.float32)
    for i, r in enumerate(res.results):
        yt = r["outt"]                                  # [224, SHARD] bf16
        out[i * SHARD : (i + 1) * SHARD, 0:HALF] = yt[0:HALF].T.astype(np.float32)
        out[i * SHARD : (i + 1) * SHARD, HALF:N] = yt[NP : NP + HALF].T.astype(
            np.float32
        )
    return out.reshape(B, T, N)
